# revision 40
# baseline (speedup 1.0000x reference)
"""Trainium2 Bass kernel: masked additive attention (B=64, T=2048, H=512).

Computation (matching the reference nn.Module):
    h      = tanh(lstm_out @ W^T + b)          # (B,T,H)
    scores = h @ v                             # (B,T)
    scores = where(mask, -inf, scores)
    attn_w = softmax(scores, axis=-1)          # fully-masked rows -> 1/T
    context = attn_w @ lstm_out                # (B,H)
    returns (context, attn_w)

Strategy (data parallel over B across 8 cores, 8 rows each):
  - Device computes, per local batch row b:
      p[t]   = exp(scores[t]) * keep[t]        (unnormalized, keep = 1-mask)
      ctx[k] = sum_t p[t] * x[t,k]             (unnormalized)
    Host divides by l = sum_t p[t] (softmax denominator) and handles the
    (never occurring in practice) fully-masked fallback.
  - h is computed in [o_part, t_free] orientation: lhsT = W^T chunks
    (stationary), rhs = x^T tiles. This makes the per-o bias a native
    per-partition ACT bias fused into the tanh, and makes scores = v . u a
    cheap PE matvec (contraction over o = partitions).
  - ctx needs contraction over t, so a second copy of x in [t_part, k_free]
    layout is shipped. Both copies are bf16 -> same total HBM bytes as one
    fp32 copy. p is transposed to partitions via a tiny DRAM bounce.
"""

import numpy as np
import ml_dtypes

_NCORES = 8
_B_TOTAL, _T, _H = 64, 2048, 512
_BL = _B_TOTAL // _NCORES  # 8 local batch rows per core
_TG = 512                  # t-group size
_G = _T // _TG             # 4 groups
_C = 4                     # 128-chunks of o/k, and ctx sub-tiles per group

_cache = {}

LAST_RESULTS = None  # BassKernelResults of the most recent device run


def _build_bass():
    import concourse.mybir as mybir
    import concourse.tile as tile
    from concourse import bacc

    dt = mybir.dt
    AF = mybir.ActivationFunctionType
    # Bacc (not raw Bass): its compile() runs generate_event_semaphores,
    # which splits multi-wait instructions to satisfy the TRN2 limit of
    # one sync wait per instruction.
    nc = bacc.Bacc("TRN2", target_bir_lowering=False)

    B, T, H, TG, G, C = _BL, _T, _H, _TG, _G, _C

    # pre-tiled on host so each group load is one fully-contiguous block
    xT = nc.dram_tensor("xT", [B, G, 128, C, TG], dt.bfloat16, kind="ExternalInput")
    x = nc.dram_tensor("x", [B, G, 128, C, H], dt.bfloat16, kind="ExternalInput")
    wT = nc.dram_tensor("wT", [128, 4, H], dt.bfloat16, kind="ExternalInput")
    bias_d = nc.dram_tensor("bias", [H], dt.float32, kind="ExternalInput")
    v_d = nc.dram_tensor("v", [H], dt.bfloat16, kind="ExternalInput")
    keep_d = nc.dram_tensor("keep", [B, T], dt.float32, kind="ExternalInput")
    p_out = nc.dram_tensor("p_out", [B, T], dt.float32, kind="ExternalOutput")
    ctx_out = nc.dram_tensor("ctx_out", [B, 128, H], dt.float32, kind="ExternalOutput")

    with tile.TileContext(nc) as tc:
        with (
            tc.tile_pool(name="statics", bufs=1) as statics,
            tc.tile_pool(name="xT_pool", bufs=6) as xT_pool,
            tc.tile_pool(name="x_pool", bufs=10) as x_pool,
            tc.tile_pool(name="u_pool", bufs=6) as u_pool,
            tc.tile_pool(name="pT_pool", bufs=8) as pT_pool,
            tc.tile_pool(name="pb_pool", bufs=2) as pb_pool,
            tc.tile_pool(name="small", bufs=2) as small,
            tc.tile_pool(name="dram_pool", bufs=8, space="DRAM") as dram_pool,
            tc.tile_pool(name="h_psum", bufs=5, space="PSUM") as h_psum,
            tc.tile_pool(name="s_psum", bufs=2, space="PSUM") as s_psum,
            tc.tile_pool(name="c_psum", bufs=1, space="PSUM") as c_psum,
        ):
            # wT first: its DMA gates the first matmul, and issuing it ahead
            # of the warm activation lets the ~2.7us ACT table load overlap
            # the transfer instead of delaying the issue.
            wT_sb = statics.tile([128, C, H], dt.bfloat16)  # [p, kc, o]
            nc.scalar.dma_start(wT_sb, wT[:])

            # Dummy activation with minimal deps: carries the one-time ACT
            # table load (walrus attaches it to the first ACTIVATE, which
            # otherwise overflows that instruction's sync-wait slots).
            warm_in = statics.tile([1, 1], dt.float32)
            nc.vector.memset(warm_in, 0.0)
            warm_out = statics.tile([1, 1], dt.float32)
            nc.scalar.activation(warm_out, warm_in, AF.Tanh)
            bias_sb = statics.tile([128, C], dt.float32)  # [p, oc] -> bias[oc*128+p]
            nc.gpsimd.dma_start(bias_sb, bias_d.rearrange("(oc p) -> p oc", p=128))
            v_sb = statics.tile([128, C], dt.bfloat16)
            nc.gpsimd.dma_start(v_sb, v_d.rearrange("(oc p) -> p oc", p=128))

            # Per-b staged tiles for the ctx matmuls, which are emitted one b
            # late (software pipelining): the p-transpose chain then has a
            # full row of slack and never stalls the PE queue.
            staged = {}

            def emit_ctx(bb):
                # 4 t-groups' partial contexts run CONCURRENTLY in distinct
                # PE column groups (rows 0/32/64/96 of one psum bank); the
                # partials are summed on the host in f32. Rows other than
                # 32*gg carry garbage and are ignored host-side.
                pT_list, x_list, pfull_b = staged.pop(bb)
                ctx_ps = c_psum.tile([128, H], dt.float32, tag="ctx", name="ctx_ps")
                # last b: col-groups 0-2 first so the PE works while group 3's
                # softmax/transpose chain (the kernel tail) completes
                phases = [(0, 1, 2), (3,)] if bb == B - 1 else [tuple(range(G))]
                for ggs in phases:
                    for j in range(C):
                        for gg in ggs:
                            nc.tensor.matmul(
                                ctx_ps[32 * gg : 32 * gg + 1, :],
                                pT_list[gg][:, j : j + 1],
                                x_list[gg][:, j],
                                start=(j == 0),
                                stop=(j == C - 1),
                                tile_position=(0, 32 * gg),
                            )
                ctx_sb = small.tile([128, H], dt.float32, tag="ctxsb", name="ctx_sb")
                nc.vector.tensor_copy(ctx_sb, ctx_ps)
                # mid-kernel outputs ride gpsimd: on sync their copy-wait
                # head-of-line-blocks the next row's input prefetch
                out_eng = nc.sync if bb == B - 1 else nc.gpsimd
                out_eng.dma_start(ctx_out[bb], ctx_sb)
                out_eng.dma_start(p_out[bb : bb + 1, :], pfull_b)

            for b in range(B):
                keep_sb = small.tile([1, T], dt.float32, tag="keep")
                nc.gpsimd.dma_start(keep_sb, keep_d[b : b + 1, :])
                pfull = pb_pool.tile([1, T], dt.float32, tag="pfull")
                pb16 = pb_pool.tile([1, T], dt.bfloat16, tag="pb16")
                pT_list = []
                x_list = []
                s_open = {}

                def finish_scores(gg):
                    # o-chunks 2/3 of scores(gg) + softmax chain, deferred one
                    # t-group so every tanh(gg) is long done (no PE stall)
                    s_ps, u_g = s_open.pop(gg)
                    for oc in (2, 3):
                        nc.tensor.matmul(
                            s_ps,
                            v_sb[:, oc : oc + 1],
                            u_g[:, oc],
                            start=False,
                            stop=(oc == C - 1),
                        )
                    tsl = slice(gg * TG, (gg + 1) * TG)
                    # p_out carries the raw exp (host applies keep); the
                    # masked bf16 for the ctx matvec is one fused mul+cast
                    nc.scalar.activation(pfull[:, tsl], s_ps, AF.Exp)
                    nc.vector.tensor_mul(pb16[:, tsl], pfull[:, tsl], keep_sb[:, tsl])
                    # transpose p (1 x TG) -> (128 x C) via DRAM bounce,
                    # matching t = p*C + j used by x_t.
                    if b < B - 1:
                        pd = dram_pool.tile([1, TG], dt.bfloat16, tag="pd", name="pd")
                        nc.gpsimd.dma_start(pd, pb16[:, tsl])
                        pT_t = pT_pool.tile(
                            [128, C], dt.bfloat16, tag="pT", name="pT_t"
                        )
                        nc.gpsimd.dma_start(
                            pT_t, pd.rearrange("one (p j) -> (one p) j", j=C)
                        )
                        pT_list.append(pT_t)

                for g in range(G):
                    xT_t = xT_pool.tile([128, C, TG], dt.bfloat16, tag="xT")
                    nc.sync.dma_start(xT_t, xT[b, g])
                    # ctx rhs tiles: t = g*TG + p*C + j (j = sub-tile)
                    x_t = x_pool.tile([128, C, H], dt.bfloat16, tag="x")
                    # b0: off the sync queue head (not needed until b1)
                    (nc.scalar if b == 0 else nc.sync).dma_start(x_t, x[b, g])

                    # h = x @ W^T in [o, t] orientation; tanh(+bias) -> u
                    u_t = u_pool.tile([128, C, TG], dt.bfloat16, tag="u")
                    for oc in range(C):
                        h_ps = h_psum.tile([128, TG], dt.float32, tag="h")
                        for kc in range(C):
                            nc.tensor.matmul(
                                h_ps,
                                wT_sb[:, kc, oc * 128 : (oc + 1) * 128],
                                xT_t[:, kc],
                                start=(kc == 0),
                                stop=(kc == C - 1),
                            )
                        nc.scalar.activation(
                            u_t[:, oc],
                            h_ps,
                            AF.Tanh,
                            bias=bias_sb[:, oc : oc + 1],
                        )

                    if g > 0:
                        finish_scores(g - 1)
                    # scores o-chunks 0/1 (their tanh is already done)
                    s_ps = s_psum.tile([1, TG], dt.float32, tag="s")
                    for oc in (0, 1):
                        nc.tensor.matmul(
                            s_ps,
                            v_sb[:, oc : oc + 1],
                            u_t[:, oc],
                            start=(oc == 0),
                            stop=False,
                        )
                    s_open[g] = (s_ps, u_t)
                    x_list.append(x_t)

                if b == B - 1:
                    # last b: bounce on the fast sync HWDGE queue (tail path;
                    # nothing queues behind). g0-2 as soon as their chains are
                    # done, g3 after its own chain below.
                    def bounce(gg):
                        # only g3 is on the critical tail path: it gets the
                        # sync queue to itself; g0-2 ride gpsimd so their
                        # issue time doesn't delay g3's bounce
                        eng = nc.sync if gg == G - 1 else nc.gpsimd
                        tsl = slice(gg * TG, (gg + 1) * TG)
                        pd = dram_pool.tile([1, TG], dt.bfloat16, tag="pd", name="pd")
                        eng.dma_start(pd, pb16[:, tsl])
                        pT_t = pT_pool.tile([128, C], dt.bfloat16, tag="pT", name="pT_t")
                        eng.dma_start(
                            pT_t, pd.rearrange("one (p j) -> (one p) j", j=C)
                        )
                        pT_list.append(pT_t)

                    for gg in range(G - 1):
                        bounce(gg)
                    finish_scores(G - 1)
                    bounce(G - 1)
                else:
                    finish_scores(G - 1)

                staged[b] = (pT_list, x_list, pfull)
                if b > 0:
                    emit_ctx(b - 1)

            emit_ctx(B - 1)

    nc.compile()
    return nc


def _get_nc():
    if "nc" not in _cache:
        _cache["nc"] = _build_bass()
    return _cache["nc"]


def _ensure_axon_profile_hook():
    """Register the NTFF profile hook that run_bass_kernel_spmd(trace=True)
    looks up via antenv.axon_hooks. This image's antenv lacks that module;
    the underlying ctypes facility in libaxon_pjrt.so exists, so shim it."""
    import sys
    import types

    try:
        from antenv.axon_hooks import get_axon_ntff_profile_hook  # noqa: F401

        return
    except ImportError:
        pass
    import antenv
    from trn_agent_boot.trn_boot import _ntff_profile_via_ctypes

    holder = {"hook": _ntff_profile_via_ctypes("/opt/axon/libaxon_pjrt.so")}
    mod = types.ModuleType("antenv.axon_hooks")
    mod.get_axon_ntff_profile_hook = lambda: holder["hook"]
    mod.set_axon_ntff_profile_hook = lambda h: holder.__setitem__("hook", h)
    sys.modules["antenv.axon_hooks"] = mod
    antenv.axon_hooks = mod


def kernel(lstm_out, padding_mask, attn_weight, attn_bias, v_weight, _trace=False):
    global LAST_RESULTS
    from concourse.bass_utils import run_bass_kernel_spmd

    if _trace:
        _ensure_axon_profile_hook()

    bf16 = ml_dtypes.bfloat16
    lstm_out = np.ascontiguousarray(np.asarray(lstm_out, dtype=np.float32))
    padding_mask = np.asarray(padding_mask)
    B, T, H = lstm_out.shape
    assert (B, T, H) == (_B_TOTAL, _T, _H)

    x_bf = lstm_out.astype(bf16)
    # x_dev[b, g, p, j, k] = x[b, g*512 + p*4 + j, k]  (t split as (g,p,j))
    x_dev = x_bf.reshape(_B_TOTAL, _G, 128, _C, _H)
    # xT_dev[b, g, p, kc, t'] = x[b, g*512 + t', kc*128 + p]
    xT_bf = np.ascontiguousarray(x_bf.transpose(0, 2, 1))  # [B, k, t]
    xT_dev = np.ascontiguousarray(
        xT_bf.reshape(_B_TOTAL, _C, 128, _G, _TG).transpose(0, 3, 2, 1, 4)
    )
    keep = (~padding_mask).astype(np.float32)
    wT = np.ascontiguousarray(
        np.asarray(attn_weight, dtype=np.float32).T.astype(bf16)
        .reshape(4, 128, _H).transpose(1, 0, 2)
    )
    bias = np.asarray(attn_bias, dtype=np.float32)
    v = np.asarray(v_weight, dtype=np.float32)[0].astype(bf16)


    nc = _get_nc()
    in_maps = []
    for c in range(_NCORES):
        sl = slice(c * _BL, (c + 1) * _BL)
        in_maps.append(
            {
                "xT": np.ascontiguousarray(xT_dev[sl]),
                "x": np.ascontiguousarray(x_dev[sl]),
                "wT": wT,
                "bias": bias,
                "v": v,
                "keep": np.ascontiguousarray(keep[sl]),
            }
        )

    try:
        res = run_bass_kernel_spmd(
            nc, in_maps, core_ids=list(range(_NCORES)), trace=_trace
        )
    except Exception:
        # one retry for transient device errors (NRT_EXEC_UNIT_UNRECOVERABLE
        # has been observed sporadically on this fabric)
        res = run_bass_kernel_spmd(
            nc, in_maps, core_ids=list(range(_NCORES)), trace=_trace
        )
    LAST_RESULTS = res

    p = np.concatenate([r["p_out"] for r in res.results], axis=0)  # (B, T) f32
    p = p * keep
    ctx4 = np.concatenate([r["ctx_out"] for r in res.results], axis=0)  # (B,128,H)
    ctx_raw = ctx4[:, 0::32, :].sum(axis=1, dtype=np.float64).astype(np.float32)

    l = p.sum(axis=-1, keepdims=True)
    dead = l == 0.0  # fully-masked rows (softmax of all -inf)
    l_safe = np.where(dead, 1.0, l)
    attn_w = p / l_safe
    context = ctx_raw / l_safe
    if dead.any():
        rows = dead[:, 0]
        attn_w[rows] = 1.0 / T
        context[rows] = lstm_out[rows].mean(axis=1)

    return (context.astype(np.float32), attn_w.astype(np.float32))


# revision 41
# speedup vs baseline: 1.0092x; 1.0092x over previous
"""Trainium2 Bass kernel: masked additive attention (B=64, T=2048, H=512).

Computation (matching the reference nn.Module):
    h      = tanh(lstm_out @ W^T + b)          # (B,T,H)
    scores = h @ v                             # (B,T)
    scores = where(mask, -inf, scores)
    attn_w = softmax(scores, axis=-1)          # fully-masked rows -> 1/T
    context = attn_w @ lstm_out                # (B,H)
    returns (context, attn_w)

Strategy (data parallel over B across 8 cores, 8 rows each):
  - Device computes, per local batch row b:
      p[t]   = exp(scores[t]) * keep[t]        (unnormalized, keep = 1-mask)
      ctx[k] = sum_t p[t] * x[t,k]             (unnormalized)
    Host divides by l = sum_t p[t] (softmax denominator) and handles the
    (never occurring in practice) fully-masked fallback.
  - h is computed in [o_part, t_free] orientation: lhsT = W^T chunks
    (stationary), rhs = x^T tiles. This makes the per-o bias a native
    per-partition ACT bias fused into the tanh, and makes scores = v . u a
    cheap PE matvec (contraction over o = partitions).
  - ctx needs contraction over t, so a second copy of x in [t_part, k_free]
    layout is shipped. Both copies are bf16 -> same total HBM bytes as one
    fp32 copy. p is transposed to partitions via a tiny DRAM bounce.
"""

import numpy as np
import ml_dtypes

_NCORES = 8
_B_TOTAL, _T, _H = 64, 2048, 512
_BL = _B_TOTAL // _NCORES  # 8 local batch rows per core
_TG = 512                  # t-group size
_G = _T // _TG             # 4 groups
_C = 4                     # 128-chunks of o/k, and ctx sub-tiles per group

_cache = {}

LAST_RESULTS = None  # BassKernelResults of the most recent device run


def _build_bass():
    import concourse.mybir as mybir
    import concourse.tile as tile
    from concourse import bacc

    dt = mybir.dt
    AF = mybir.ActivationFunctionType
    # Bacc (not raw Bass): its compile() runs generate_event_semaphores,
    # which splits multi-wait instructions to satisfy the TRN2 limit of
    # one sync wait per instruction.
    nc = bacc.Bacc("TRN2", target_bir_lowering=False)

    B, T, H, TG, G, C = _BL, _T, _H, _TG, _G, _C

    # pre-tiled on host so each group load is one fully-contiguous block
    xT = nc.dram_tensor("xT", [B, G, 128, C, TG], dt.bfloat16, kind="ExternalInput")
    x = nc.dram_tensor("x", [B, G, 128, C, H], dt.bfloat16, kind="ExternalInput")
    wT = nc.dram_tensor("wT", [128, 4, H], dt.bfloat16, kind="ExternalInput")
    bias_d = nc.dram_tensor("bias", [H], dt.float32, kind="ExternalInput")
    v_d = nc.dram_tensor("v", [H], dt.bfloat16, kind="ExternalInput")
    keep_d = nc.dram_tensor("keep", [B, T], dt.float32, kind="ExternalInput")
    p_out = nc.dram_tensor("p_out", [B, T], dt.float32, kind="ExternalOutput")
    ctx_out = nc.dram_tensor("ctx_out", [B, 128, H], dt.float32, kind="ExternalOutput")

    with tile.TileContext(nc) as tc:
        with (
            tc.tile_pool(name="statics", bufs=1) as statics,
            tc.tile_pool(name="xT_pool", bufs=6) as xT_pool,
            tc.tile_pool(name="x_pool", bufs=10) as x_pool,
            tc.tile_pool(name="u_pool", bufs=6) as u_pool,
            tc.tile_pool(name="pT_pool", bufs=8) as pT_pool,
            tc.tile_pool(name="pb_pool", bufs=2) as pb_pool,
            tc.tile_pool(name="small", bufs=2) as small,
            tc.tile_pool(name="dram_pool", bufs=8, space="DRAM") as dram_pool,
            tc.tile_pool(name="h_psum", bufs=5, space="PSUM") as h_psum,
            tc.tile_pool(name="s_psum", bufs=2, space="PSUM") as s_psum,
            tc.tile_pool(name="c_psum", bufs=1, space="PSUM") as c_psum,
        ):
            # wT first: its DMA gates the first matmul, and issuing it ahead
            # of the warm activation lets the ~2.7us ACT table load overlap
            # the transfer instead of delaying the issue.
            wT_sb = statics.tile([128, C, H], dt.bfloat16)  # [p, kc, o]
            nc.scalar.dma_start(wT_sb, wT[:])

            # Dummy activation with minimal deps: carries the one-time ACT
            # table load (walrus attaches it to the first ACTIVATE, which
            # otherwise overflows that instruction's sync-wait slots).
            warm_in = statics.tile([1, 1], dt.float32)
            nc.vector.memset(warm_in, 0.0)
            warm_out = statics.tile([1, 1], dt.float32)
            nc.scalar.activation(warm_out, warm_in, AF.Tanh)
            bias_sb = statics.tile([128, C], dt.float32)  # [p, oc] -> bias[oc*128+p]
            nc.gpsimd.dma_start(bias_sb, bias_d.rearrange("(oc p) -> p oc", p=128))
            v_sb = statics.tile([128, C], dt.bfloat16)
            nc.gpsimd.dma_start(v_sb, v_d.rearrange("(oc p) -> p oc", p=128))

            # Per-b staged tiles for the ctx matmuls, which are emitted one b
            # late (software pipelining): the p-transpose chain then has a
            # full row of slack and never stalls the PE queue.
            staged = {}

            def emit_ctx(bb):
                # 4 t-groups' partial contexts run CONCURRENTLY in distinct
                # PE column groups (rows 0/32/64/96 of one psum bank); the
                # partials are summed on the host in f32. Rows other than
                # 32*gg carry garbage and are ignored host-side.
                pT_list, x_list, pfull_b = staged.pop(bb)
                ctx_ps = c_psum.tile([128, H], dt.float32, tag="ctx", name="ctx_ps")
                # last b: col-groups 0-2 first so the PE works while group 3's
                # softmax/transpose chain (the kernel tail) completes
                phases = [(0, 1, 2), (3,)] if bb == B - 1 else [tuple(range(G))]
                for ggs in phases:
                    for j in range(C):
                        for gg in ggs:
                            nc.tensor.matmul(
                                ctx_ps[32 * gg : 32 * gg + 1, :],
                                pT_list[gg][:, j : j + 1],
                                x_list[gg][:, j],
                                start=(j == 0),
                                stop=(j == C - 1),
                                tile_position=(0, 32 * gg),
                            )
                ctx_sb = small.tile([128, H], dt.float32, tag="ctxsb", name="ctx_sb")
                nc.vector.tensor_copy(ctx_sb, ctx_ps)
                # mid-kernel outputs ride gpsimd: on sync their copy-wait
                # head-of-line-blocks the next row's input prefetch
                out_eng = nc.sync if bb == B - 1 else nc.gpsimd
                out_eng.dma_start(ctx_out[bb], ctx_sb)
                out_eng.dma_start(p_out[bb : bb + 1, :], pfull_b)

            for b in range(B):
                keep_sb = small.tile([1, T], dt.float32, tag="keep")
                nc.gpsimd.dma_start(keep_sb, keep_d[b : b + 1, :])
                pfull = pb_pool.tile([1, T], dt.float32, tag="pfull")
                pb16 = pb_pool.tile([1, T], dt.bfloat16, tag="pb16")
                pT_list = []
                x_list = []
                s_open = {}

                def finish_scores(gg):
                    # o-chunks 2/3 of scores(gg) + softmax chain, deferred one
                    # t-group so every tanh(gg) is long done (no PE stall)
                    s_ps, u_g = s_open.pop(gg)
                    for oc in (2, 3):
                        nc.tensor.matmul(
                            s_ps,
                            v_sb[:, oc : oc + 1],
                            u_g[:, oc],
                            start=False,
                            stop=(oc == C - 1),
                        )
                    tsl = slice(gg * TG, (gg + 1) * TG)
                    # p_out carries the raw exp (host applies keep); the
                    # masked bf16 for the ctx matvec is one fused mul+cast
                    nc.scalar.activation(pfull[:, tsl], s_ps, AF.Exp)
                    nc.vector.tensor_mul(pb16[:, tsl], pfull[:, tsl], keep_sb[:, tsl])
                    # transpose p (1 x TG) -> (128 x C) via DRAM bounce,
                    # matching t = p*C + j used by x_t.
                    if b < B - 1:
                        pd = dram_pool.tile([1, TG], dt.bfloat16, tag="pd", name="pd")
                        nc.gpsimd.dma_start(pd, pb16[:, tsl])
                        pT_t = pT_pool.tile(
                            [128, C], dt.bfloat16, tag="pT", name="pT_t"
                        )
                        nc.gpsimd.dma_start(
                            pT_t, pd.rearrange("one (p j) -> (one p) j", j=C)
                        )
                        pT_list.append(pT_t)

                for g in range(G):
                    xT_t = xT_pool.tile([128, C, TG], dt.bfloat16, tag="xT")
                    nc.sync.dma_start(xT_t, xT[b, g])
                    # ctx rhs tiles: t = g*TG + p*C + j (j = sub-tile)
                    x_t = x_pool.tile([128, C, H], dt.bfloat16, tag="x")
                    # b0: off the sync queue head (not needed until b1)
                    (nc.scalar if b == 0 else nc.sync).dma_start(x_t, x[b, g])

                    # h = x @ W^T in [o, t] orientation; tanh(+bias) -> u
                    u_t = u_pool.tile([128, C, TG], dt.bfloat16, tag="u")
                    for oc in range(C):
                        h_ps = h_psum.tile([128, TG], dt.float32, tag="h")
                        for kc in range(C):
                            nc.tensor.matmul(
                                h_ps,
                                wT_sb[:, kc, oc * 128 : (oc + 1) * 128],
                                xT_t[:, kc],
                                start=(kc == 0),
                                stop=(kc == C - 1),
                            )
                        nc.scalar.activation(
                            u_t[:, oc],
                            h_ps,
                            AF.Tanh,
                            bias=bias_sb[:, oc : oc + 1],
                        )

                    if g > 0:
                        finish_scores(g - 1)
                    # scores o-chunks 0/1 (their tanh is already done)
                    s_ps = s_psum.tile([1, TG], dt.float32, tag="s")
                    for oc in (0, 1):
                        nc.tensor.matmul(
                            s_ps,
                            v_sb[:, oc : oc + 1],
                            u_t[:, oc],
                            start=(oc == 0),
                            stop=False,
                        )
                    s_open[g] = (s_ps, u_t)
                    x_list.append(x_t)

                if b == B - 1:
                    # last b: bounce on the fast sync HWDGE queue (tail path;
                    # nothing queues behind). g0-2 as soon as their chains are
                    # done, g3 after its own chain below.
                    def bounce(gg):
                        # only g3 is on the critical tail path: it gets the
                        # sync queue to itself; g0-2 ride the (idle, HWDGE)
                        # scalar queue so their issue time doesn't delay g3
                        eng = nc.sync if gg == G - 1 else nc.scalar
                        tsl = slice(gg * TG, (gg + 1) * TG)
                        pd = dram_pool.tile([1, TG], dt.bfloat16, tag="pd", name="pd")
                        eng.dma_start(pd, pb16[:, tsl])
                        pT_t = pT_pool.tile([128, C], dt.bfloat16, tag="pT", name="pT_t")
                        eng.dma_start(
                            pT_t, pd.rearrange("one (p j) -> (one p) j", j=C)
                        )
                        pT_list.append(pT_t)

                    for gg in range(G - 1):
                        bounce(gg)
                    finish_scores(G - 1)
                    bounce(G - 1)
                else:
                    finish_scores(G - 1)

                staged[b] = (pT_list, x_list, pfull)
                if b > 0:
                    emit_ctx(b - 1)

            emit_ctx(B - 1)

    nc.compile()
    return nc


def _get_nc():
    if "nc" not in _cache:
        _cache["nc"] = _build_bass()
    return _cache["nc"]


def _ensure_axon_profile_hook():
    """Register the NTFF profile hook that run_bass_kernel_spmd(trace=True)
    looks up via antenv.axon_hooks. This image's antenv lacks that module;
    the underlying ctypes facility in libaxon_pjrt.so exists, so shim it."""
    import sys
    import types

    try:
        from antenv.axon_hooks import get_axon_ntff_profile_hook  # noqa: F401

        return
    except ImportError:
        pass
    import antenv
    from trn_agent_boot.trn_boot import _ntff_profile_via_ctypes

    holder = {"hook": _ntff_profile_via_ctypes("/opt/axon/libaxon_pjrt.so")}
    mod = types.ModuleType("antenv.axon_hooks")
    mod.get_axon_ntff_profile_hook = lambda: holder["hook"]
    mod.set_axon_ntff_profile_hook = lambda h: holder.__setitem__("hook", h)
    sys.modules["antenv.axon_hooks"] = mod
    antenv.axon_hooks = mod


def kernel(lstm_out, padding_mask, attn_weight, attn_bias, v_weight, _trace=False):
    global LAST_RESULTS
    from concourse.bass_utils import run_bass_kernel_spmd

    if _trace:
        _ensure_axon_profile_hook()

    bf16 = ml_dtypes.bfloat16
    lstm_out = np.ascontiguousarray(np.asarray(lstm_out, dtype=np.float32))
    padding_mask = np.asarray(padding_mask)
    B, T, H = lstm_out.shape
    assert (B, T, H) == (_B_TOTAL, _T, _H)

    x_bf = lstm_out.astype(bf16)
    # x_dev[b, g, p, j, k] = x[b, g*512 + p*4 + j, k]  (t split as (g,p,j))
    x_dev = x_bf.reshape(_B_TOTAL, _G, 128, _C, _H)
    # xT_dev[b, g, p, kc, t'] = x[b, g*512 + t', kc*128 + p]
    xT_bf = np.ascontiguousarray(x_bf.transpose(0, 2, 1))  # [B, k, t]
    xT_dev = np.ascontiguousarray(
        xT_bf.reshape(_B_TOTAL, _C, 128, _G, _TG).transpose(0, 3, 2, 1, 4)
    )
    keep = (~padding_mask).astype(np.float32)
    wT = np.ascontiguousarray(
        np.asarray(attn_weight, dtype=np.float32).T.astype(bf16)
        .reshape(4, 128, _H).transpose(1, 0, 2)
    )
    bias = np.asarray(attn_bias, dtype=np.float32)
    v = np.asarray(v_weight, dtype=np.float32)[0].astype(bf16)


    nc = _get_nc()
    in_maps = []
    for c in range(_NCORES):
        sl = slice(c * _BL, (c + 1) * _BL)
        in_maps.append(
            {
                "xT": np.ascontiguousarray(xT_dev[sl]),
                "x": np.ascontiguousarray(x_dev[sl]),
                "wT": wT,
                "bias": bias,
                "v": v,
                "keep": np.ascontiguousarray(keep[sl]),
            }
        )

    try:
        res = run_bass_kernel_spmd(
            nc, in_maps, core_ids=list(range(_NCORES)), trace=_trace
        )
    except Exception:
        # one retry for transient device errors (NRT_EXEC_UNIT_UNRECOVERABLE
        # has been observed sporadically on this fabric)
        res = run_bass_kernel_spmd(
            nc, in_maps, core_ids=list(range(_NCORES)), trace=_trace
        )
    LAST_RESULTS = res

    p = np.concatenate([r["p_out"] for r in res.results], axis=0)  # (B, T) f32
    p = p * keep
    ctx4 = np.concatenate([r["ctx_out"] for r in res.results], axis=0)  # (B,128,H)
    ctx_raw = ctx4[:, 0::32, :].sum(axis=1, dtype=np.float64).astype(np.float32)

    l = p.sum(axis=-1, keepdims=True)
    dead = l == 0.0  # fully-masked rows (softmax of all -inf)
    l_safe = np.where(dead, 1.0, l)
    attn_w = p / l_safe
    context = ctx_raw / l_safe
    if dead.any():
        rows = dead[:, 0]
        attn_w[rows] = 1.0 / T
        context[rows] = lstm_out[rows].mean(axis=1)

    return (context.astype(np.float32), attn_w.astype(np.float32))


# revision 42
# speedup vs baseline: 1.0262x; 1.0168x over previous
"""Trainium2 Bass kernel: masked additive attention (B=64, T=2048, H=512).

Computation (matching the reference nn.Module):
    h      = tanh(lstm_out @ W^T + b)          # (B,T,H)
    scores = h @ v                             # (B,T)
    scores = where(mask, -inf, scores)
    attn_w = softmax(scores, axis=-1)          # fully-masked rows -> 1/T
    context = attn_w @ lstm_out                # (B,H)
    returns (context, attn_w)

Strategy (data parallel over B across 8 cores, 8 rows each):
  - Device computes, per local batch row b:
      p[t]   = exp(scores[t]) * keep[t]        (unnormalized, keep = 1-mask)
      ctx[k] = sum_t p[t] * x[t,k]             (unnormalized)
    Host divides by l = sum_t p[t] (softmax denominator) and handles the
    (never occurring in practice) fully-masked fallback.
  - h is computed in [o_part, t_free] orientation: lhsT = W^T chunks
    (stationary), rhs = x^T tiles. This makes the per-o bias a native
    per-partition ACT bias fused into the tanh, and makes scores = v . u a
    cheap PE matvec (contraction over o = partitions).
  - ctx needs contraction over t, so a second copy of x in [t_part, k_free]
    layout is shipped. Both copies are bf16 -> same total HBM bytes as one
    fp32 copy. p is transposed to partitions via a tiny DRAM bounce.
"""

import numpy as np
import ml_dtypes

_NCORES = 8
_B_TOTAL, _T, _H = 64, 2048, 512
_BL = _B_TOTAL // _NCORES  # 8 local batch rows per core
_TG = 512                  # t-group size
_G = _T // _TG             # 4 groups
_C = 4                     # 128-chunks of o/k, and ctx sub-tiles per group

_cache = {}

LAST_RESULTS = None  # BassKernelResults of the most recent device run


def _build_bass():
    import concourse.mybir as mybir
    import concourse.tile as tile
    from concourse import bacc

    dt = mybir.dt
    AF = mybir.ActivationFunctionType
    # Bacc (not raw Bass): its compile() runs generate_event_semaphores,
    # which splits multi-wait instructions to satisfy the TRN2 limit of
    # one sync wait per instruction.
    nc = bacc.Bacc("TRN2", target_bir_lowering=False)

    B, T, H, TG, G, C = _BL, _T, _H, _TG, _G, _C

    # pre-tiled on host so each group load is one fully-contiguous block
    xT = nc.dram_tensor("xT", [B, G, 128, C, TG], dt.bfloat16, kind="ExternalInput")
    x = nc.dram_tensor("x", [B, G, 128, C, H], dt.bfloat16, kind="ExternalInput")
    wT = nc.dram_tensor("wT", [128, 4, H], dt.bfloat16, kind="ExternalInput")
    bias_d = nc.dram_tensor("bias", [H], dt.float32, kind="ExternalInput")
    v_d = nc.dram_tensor("v", [H], dt.bfloat16, kind="ExternalInput")
    keep_d = nc.dram_tensor("keep", [B, T], dt.float32, kind="ExternalInput")
    p_out = nc.dram_tensor("p_out", [B, T], dt.float32, kind="ExternalOutput")
    ctx_out = nc.dram_tensor("ctx_out", [B, 128, H], dt.float32, kind="ExternalOutput")

    with tile.TileContext(nc) as tc:
        with (
            tc.tile_pool(name="statics", bufs=1) as statics,
            tc.tile_pool(name="xT_pool", bufs=6) as xT_pool,
            tc.tile_pool(name="x_pool", bufs=10) as x_pool,
            tc.tile_pool(name="u_pool", bufs=6) as u_pool,
            tc.tile_pool(name="pT_pool", bufs=8) as pT_pool,
            tc.tile_pool(name="pb_pool", bufs=2) as pb_pool,
            tc.tile_pool(name="small", bufs=2) as small,
            tc.tile_pool(name="dram_pool", bufs=8, space="DRAM") as dram_pool,
            tc.tile_pool(name="h_psum", bufs=4, space="PSUM") as h_psum,
            tc.tile_pool(name="s_psum", bufs=2, space="PSUM") as s_psum,
            tc.tile_pool(name="c_psum", bufs=2, space="PSUM") as c_psum,
        ):
            # wT first: its DMA gates the first matmul, and issuing it ahead
            # of the warm activation lets the ~2.7us ACT table load overlap
            # the transfer instead of delaying the issue.
            wT_sb = statics.tile([128, C, H], dt.bfloat16)  # [p, kc, o]
            nc.scalar.dma_start(wT_sb, wT[:])

            # Dummy activation with minimal deps: carries the one-time ACT
            # table load (walrus attaches it to the first ACTIVATE, which
            # otherwise overflows that instruction's sync-wait slots).
            warm_in = statics.tile([1, 1], dt.float32)
            nc.vector.memset(warm_in, 0.0)
            warm_out = statics.tile([1, 1], dt.float32)
            nc.scalar.activation(warm_out, warm_in, AF.Tanh)
            bias_sb = statics.tile([128, C], dt.float32)  # [p, oc] -> bias[oc*128+p]
            nc.gpsimd.dma_start(bias_sb, bias_d.rearrange("(oc p) -> p oc", p=128))
            v_sb = statics.tile([128, C], dt.bfloat16)
            nc.gpsimd.dma_start(v_sb, v_d.rearrange("(oc p) -> p oc", p=128))

            # Per-b staged tiles for the ctx matmuls, which are emitted one b
            # late (software pipelining): the p-transpose chain then has a
            # full row of slack and never stalls the PE queue.
            staged = {}

            def emit_ctx(bb):
                # 4 t-groups' partial contexts run CONCURRENTLY in distinct
                # PE column groups (rows 0/32/64/96 of one psum bank); the
                # partials are summed on the host in f32. Rows other than
                # 32*gg carry garbage and are ignored host-side.
                pT_list, x_list, pfull_b = staged.pop(bb)
                ctx_ps = c_psum.tile([128, H], dt.float32, tag="ctx", name="ctx_ps")
                # last b: col-groups 0-2 first so the PE works while group 3's
                # softmax/transpose chain (the kernel tail) completes
                phases = [(0, 1, 2), (3,)] if bb == B - 1 else [tuple(range(G))]
                for ggs in phases:
                    for j in range(C):
                        for gg in ggs:
                            nc.tensor.matmul(
                                ctx_ps[32 * gg : 32 * gg + 1, :],
                                pT_list[gg][:, j : j + 1],
                                x_list[gg][:, j],
                                start=(j == 0),
                                stop=(j == C - 1),
                                tile_position=(0, 32 * gg),
                            )
                ctx_sb = small.tile([128, H], dt.float32, tag="ctxsb", name="ctx_sb")
                nc.vector.tensor_copy(ctx_sb, ctx_ps)
                # mid-kernel outputs ride gpsimd: on sync their copy-wait
                # head-of-line-blocks the next row's input prefetch
                out_eng = nc.sync if bb == B - 1 else nc.gpsimd
                out_eng.dma_start(ctx_out[bb], ctx_sb)
                out_eng.dma_start(p_out[bb : bb + 1, :], pfull_b)

            for b in range(B):
                keep_sb = small.tile([1, T], dt.float32, tag="keep")
                nc.gpsimd.dma_start(keep_sb, keep_d[b : b + 1, :])
                pfull = pb_pool.tile([1, T], dt.float32, tag="pfull")
                pb16 = pb_pool.tile([1, T], dt.bfloat16, tag="pb16")
                pT_list = []
                x_list = []
                s_open = {}

                def finish_scores(gg):
                    # o-chunks 2/3 of scores(gg) + softmax chain, deferred one
                    # t-group so every tanh(gg) is long done (no PE stall)
                    s_ps, u_g = s_open.pop(gg)
                    for oc in (2, 3):
                        nc.tensor.matmul(
                            s_ps,
                            v_sb[:, oc : oc + 1],
                            u_g[:, oc],
                            start=False,
                            stop=(oc == C - 1),
                        )
                    tsl = slice(gg * TG, (gg + 1) * TG)
                    # p_out carries the raw exp (host applies keep); the
                    # masked bf16 for the ctx matvec is one fused mul+cast
                    nc.scalar.activation(pfull[:, tsl], s_ps, AF.Exp)
                    nc.vector.tensor_mul(pb16[:, tsl], pfull[:, tsl], keep_sb[:, tsl])
                    # transpose p (1 x TG) -> (128 x C) via DRAM bounce,
                    # matching t = p*C + j used by x_t.
                    if b < B - 1:
                        pd = dram_pool.tile([1, TG], dt.bfloat16, tag="pd", name="pd")
                        nc.gpsimd.dma_start(pd, pb16[:, tsl])
                        pT_t = pT_pool.tile(
                            [128, C], dt.bfloat16, tag="pT", name="pT_t"
                        )
                        nc.gpsimd.dma_start(
                            pT_t, pd.rearrange("one (p j) -> (one p) j", j=C)
                        )
                        pT_list.append(pT_t)

                for g in range(G):
                    xT_t = xT_pool.tile([128, C, TG], dt.bfloat16, tag="xT")
                    nc.sync.dma_start(xT_t, xT[b, g])
                    # ctx rhs tiles: t = g*TG + p*C + j (j = sub-tile)
                    x_t = x_pool.tile([128, C, H], dt.bfloat16, tag="x")
                    # b0: off the sync queue head (not needed until b1)
                    (nc.scalar if b == 0 else nc.sync).dma_start(x_t, x[b, g])

                    # h = x @ W^T in [o, t] orientation; tanh(+bias) -> u
                    u_t = u_pool.tile([128, C, TG], dt.bfloat16, tag="u")
                    for oc in range(C):
                        h_ps = h_psum.tile([128, TG], dt.float32, tag="h")
                        for kc in range(C):
                            nc.tensor.matmul(
                                h_ps,
                                wT_sb[:, kc, oc * 128 : (oc + 1) * 128],
                                xT_t[:, kc],
                                start=(kc == 0),
                                stop=(kc == C - 1),
                            )
                        nc.scalar.activation(
                            u_t[:, oc],
                            h_ps,
                            AF.Tanh,
                            bias=bias_sb[:, oc : oc + 1],
                        )

                    if g > 0:
                        finish_scores(g - 1)
                    # scores o-chunks 0/1 (their tanh is already done)
                    s_ps = s_psum.tile([1, TG], dt.float32, tag="s")
                    for oc in (0, 1):
                        nc.tensor.matmul(
                            s_ps,
                            v_sb[:, oc : oc + 1],
                            u_t[:, oc],
                            start=(oc == 0),
                            stop=False,
                        )
                    s_open[g] = (s_ps, u_t)
                    x_list.append(x_t)

                if b == B - 1:
                    # last b: bounce on the fast sync HWDGE queue (tail path;
                    # nothing queues behind). g0-2 as soon as their chains are
                    # done, g3 after its own chain below.
                    def bounce(gg):
                        # only g3 is on the critical tail path: it gets the
                        # sync queue to itself; g0-2 ride the (idle, HWDGE)
                        # scalar queue so their issue time doesn't delay g3
                        eng = nc.sync if gg == G - 1 else nc.scalar
                        tsl = slice(gg * TG, (gg + 1) * TG)
                        pd = dram_pool.tile([1, TG], dt.bfloat16, tag="pd", name="pd")
                        eng.dma_start(pd, pb16[:, tsl])
                        pT_t = pT_pool.tile([128, C], dt.bfloat16, tag="pT", name="pT_t")
                        eng.dma_start(
                            pT_t, pd.rearrange("one (p j) -> (one p) j", j=C)
                        )
                        pT_list.append(pT_t)

                    for gg in range(G - 1):
                        bounce(gg)
                    finish_scores(G - 1)
                    bounce(G - 1)
                else:
                    finish_scores(G - 1)

                staged[b] = (pT_list, x_list, pfull)
                if b > 0:
                    emit_ctx(b - 1)

            emit_ctx(B - 1)

    nc.compile()
    return nc


def _get_nc():
    if "nc" not in _cache:
        _cache["nc"] = _build_bass()
    return _cache["nc"]


def _ensure_axon_profile_hook():
    """Register the NTFF profile hook that run_bass_kernel_spmd(trace=True)
    looks up via antenv.axon_hooks. This image's antenv lacks that module;
    the underlying ctypes facility in libaxon_pjrt.so exists, so shim it."""
    import sys
    import types

    try:
        from antenv.axon_hooks import get_axon_ntff_profile_hook  # noqa: F401

        return
    except ImportError:
        pass
    import antenv
    from trn_agent_boot.trn_boot import _ntff_profile_via_ctypes

    holder = {"hook": _ntff_profile_via_ctypes("/opt/axon/libaxon_pjrt.so")}
    mod = types.ModuleType("antenv.axon_hooks")
    mod.get_axon_ntff_profile_hook = lambda: holder["hook"]
    mod.set_axon_ntff_profile_hook = lambda h: holder.__setitem__("hook", h)
    sys.modules["antenv.axon_hooks"] = mod
    antenv.axon_hooks = mod


def kernel(lstm_out, padding_mask, attn_weight, attn_bias, v_weight, _trace=False):
    global LAST_RESULTS
    from concourse.bass_utils import run_bass_kernel_spmd

    if _trace:
        _ensure_axon_profile_hook()

    bf16 = ml_dtypes.bfloat16
    lstm_out = np.ascontiguousarray(np.asarray(lstm_out, dtype=np.float32))
    padding_mask = np.asarray(padding_mask)
    B, T, H = lstm_out.shape
    assert (B, T, H) == (_B_TOTAL, _T, _H)

    x_bf = lstm_out.astype(bf16)
    # x_dev[b, g, p, j, k] = x[b, g*512 + p*4 + j, k]  (t split as (g,p,j))
    x_dev = x_bf.reshape(_B_TOTAL, _G, 128, _C, _H)
    # xT_dev[b, g, p, kc, t'] = x[b, g*512 + t', kc*128 + p]
    xT_bf = np.ascontiguousarray(x_bf.transpose(0, 2, 1))  # [B, k, t]
    xT_dev = np.ascontiguousarray(
        xT_bf.reshape(_B_TOTAL, _C, 128, _G, _TG).transpose(0, 3, 2, 1, 4)
    )
    keep = (~padding_mask).astype(np.float32)
    wT = np.ascontiguousarray(
        np.asarray(attn_weight, dtype=np.float32).T.astype(bf16)
        .reshape(4, 128, _H).transpose(1, 0, 2)
    )
    bias = np.asarray(attn_bias, dtype=np.float32)
    v = np.asarray(v_weight, dtype=np.float32)[0].astype(bf16)


    nc = _get_nc()
    in_maps = []
    for c in range(_NCORES):
        sl = slice(c * _BL, (c + 1) * _BL)
        in_maps.append(
            {
                "xT": np.ascontiguousarray(xT_dev[sl]),
                "x": np.ascontiguousarray(x_dev[sl]),
                "wT": wT,
                "bias": bias,
                "v": v,
                "keep": np.ascontiguousarray(keep[sl]),
            }
        )

    try:
        res = run_bass_kernel_spmd(
            nc, in_maps, core_ids=list(range(_NCORES)), trace=_trace
        )
    except Exception:
        # one retry for transient device errors (NRT_EXEC_UNIT_UNRECOVERABLE
        # has been observed sporadically on this fabric)
        res = run_bass_kernel_spmd(
            nc, in_maps, core_ids=list(range(_NCORES)), trace=_trace
        )
    LAST_RESULTS = res

    p = np.concatenate([r["p_out"] for r in res.results], axis=0)  # (B, T) f32
    p = p * keep
    ctx4 = np.concatenate([r["ctx_out"] for r in res.results], axis=0)  # (B,128,H)
    ctx_raw = ctx4[:, 0::32, :].sum(axis=1, dtype=np.float64).astype(np.float32)

    l = p.sum(axis=-1, keepdims=True)
    dead = l == 0.0  # fully-masked rows (softmax of all -inf)
    l_safe = np.where(dead, 1.0, l)
    attn_w = p / l_safe
    context = ctx_raw / l_safe
    if dead.any():
        rows = dead[:, 0]
        attn_w[rows] = 1.0 / T
        context[rows] = lstm_out[rows].mean(axis=1)

    return (context.astype(np.float32), attn_w.astype(np.float32))
